# revision 2
# baseline (speedup 1.0000x reference)
"""Trainium2 Bass kernel for nn_AttentionLayer (B=16, S=2048, D=512, H=64).

Data-parallel over batch: 8 NeuronCores x 2 batch items each; no collectives.

Math (per batch item b):
  qT = (Wq^T x^T + bq)/sqrt(H);  kT = Wk^T x^T + bk      [64, S] each
  w = exp(qT^T kT)  (no rowmax pass: |scores| <= ~9)
  out[h] = sum_t cbar[t] * x[t, :] @ Wv / S + bv, cbar = sum_s w[s,:]/Z[s]
  (V is never materialized.)

Structure:
  - fused projection stationary [Wq/sqrt(H) | Wk] -> A = [qT;kT] (32 MMs
    per batch); B = [kT;qT] via one PE permutation matmul per chunk.
  - scores row-strips [128,2048] via row-packed matmul pairs
    (tile_position (0,0)/(64,0)) streaming two column chunks at once.
  - exp SPLIT between ScalarE and VectorE per row-strip:
      * ScalarE strips: activation Exp (f32 PSUM -> bf16 SBUF) with
        accum_out rowsum.
      * DVE strips: Schraudolph bit-trick (i16 = s*log2e*128 + 16250.9 ==
        bf16 bits of exp(s)) via tensor_scalar f32 PSUM -> i16 SBUF.
        The colsum matmul reads the i16 tile DIRECTLY via .bitcast(bf16)
        (PE reads of bitcast APs are safe on HW - measured; DVE/ScalarE
        compute ops on bitcast APs crash). Rowsum needs a real bf16 tile:
        one SWDGE byte-copy (gpsimd queue) + one in-place 4x tensor_scalar
        pass with accum_out.
      Per-row softmax normalization cancels the Schraudolph per-element
      bias exactly (each row lives entirely on one engine).
  - colsum in groups of 4 strips (batched reciprocal), col-packed
    (0,32c) accumulation into one psum bank.
  - epilogue without DRAM round-trips: cbar transposed via 4 PE
    transposes (f32 identity), g transposed via 4 K=1 matmuls.
  - prologue: SWDGE cast HBM f32 -> SBUF xn bf16, bounce to DRAM (scalar
    HWDGE queue; sync queue carries only transposes), contiguous-read DMA
    transposes in per-half groups; batch-1 casts held on batch-0
    transposes (transpose-mode switches trigger a global DMA drain); PE
    warmup matmuls hold HAM at full clock during the lead-in; exp ACT
    table preloaded during the prologue.

Known landmines (measured on HW):
  - bitcast APs on DVE/ScalarE tensor ops -> NRT_EXEC_UNIT_UNRECOVERABLE.
    PE matmul rhs and DMA APs are safe.
  - SWDGE DRAM->DRAM cast DMA -> INTERNAL crash.
  - gpsimd tensor_scalar/scalar_tensor_tensor with accum_out fails to
    compile; accum_out needs op1 set.
  - dma_start_transpose requires contiguous last-dim input and 2-byte
    dtype (hence the DRAM bounce for xT).
  - interleaving SBUF->SBUF copies with DMA transposes on the Sync queue
    causes multi-us mode-switch drains.
"""

import os as _os

import numpy as np

B, S, D, H = 16, 2048, 512, 64
NCORES = 8
BPC = B // NCORES  # batches per core
P = 128
NT = S // P  # 16 row strips
ND = D // P  # 4 d tiles
NC4 = S // 512  # 4 column chunks of 512

# Row strips whose exp runs on VectorE (Schraudolph); rest on ScalarE.
# Default: odd strips (alternating engines pipelines Scalar/DVE).
if _os.environ.get("K_DVE_EXP"):
    DVE_TILES = tuple(int(t) for t in _os.environ["K_DVE_EXP"].split(","))
else:
    _n = int(_os.environ.get("K_DVE_N", "7"))
    _per = (1, 3, 5, 7, 9, 11, 13, 15)[:_n]
    DVE_TILES = tuple(b * 16 + i for b in range(BPC) for i in _per)

# Schraudolph constants for bf16-bits exp: i16 = s * A16 + B16
A16 = float(np.log2(np.e) * 128.0)
B16 = 16250.91

N_WARMUP_MM = int(_os.environ.get("K_WARM", "112"))


def build_nc():
    import concourse.bacc as bacc
    import concourse.mybir as mybir
    import concourse.tile as tile
    from concourse.tile_rust import add_dep_helper

    f32 = mybir.dt.float32
    bf16 = mybir.dt.bfloat16
    i16 = mybir.dt.int16
    Exp = mybir.ActivationFunctionType.Exp
    Identity = mybir.ActivationFunctionType.Identity
    Copy = mybir.ActivationFunctionType.Copy
    X = mybir.AxisListType.X
    add = mybir.AluOpType.add
    mult = mybir.AluOpType.mult

    nc = bacc.Bacc("TRN2", target_bir_lowering=False)

    x_ext = nc.declare_dram_parameter("inputs", [BPC, S, D], f32, isOutput=False)
    wq_ext = nc.declare_dram_parameter("Wq", [D, H], f32, isOutput=False)
    bq_ext = nc.declare_dram_parameter("bq", [H], f32, isOutput=False)
    wk_ext = nc.declare_dram_parameter("Wk", [D, H], f32, isOutput=False)
    bk_ext = nc.declare_dram_parameter("bk", [H], f32, isOutput=False)
    wv_ext = nc.declare_dram_parameter("Wv", [D, H], f32, isOutput=False)
    bv_ext = nc.declare_dram_parameter("bv", [H], f32, isOutput=False)
    ident_ext = nc.declare_dram_parameter("ident128", [P, P], f32, isOutput=False)
    perm_ext = nc.declare_dram_parameter("perm64", [P, P], f32, isOutput=False)
    out_ext = nc.declare_dram_parameter("out", [BPC, H], f32, isOutput=True)

    inv_sqrt_h = 1.0 / float(np.sqrt(H))

    with tile.TileContext(nc) as tc:
        with (
            tc.tile_pool(name="singles", bufs=1) as singles,
            tc.tile_pool(name="xn", bufs=8) as xn_pool,
            tc.tile_pool(name="xT", bufs=16) as xT_pool,
            tc.tile_pool(name="qkT", bufs=4) as qkT_pool,
            tc.tile_pool(name="w", bufs=8) as w_pool,
            tc.tile_pool(name="sc16", bufs=7) as sc_pool,
            tc.tile_pool(name="wb", bufs=3) as wb_pool,
            tc.tile_pool(name="zr", bufs=6) as zr_pool,
            tc.tile_pool(name="misc", bufs=4) as misc_pool,
            tc.tile_pool(name="dram", bufs=8, space="DRAM") as dram_pool,
            tc.tile_pool(name="mm", bufs=3, space="PSUM") as mm_pool,
            tc.tile_pool(name="col", bufs=1, space="PSUM") as col_pool,
            tc.tile_pool(name="epi", bufs=1, space="PSUM") as epi_pool,
        ):
            # ---- constants / weights prep (once); scalar HWDGE queue ----
            ident_sb = singles.tile([P, P], f32)
            nc.scalar.dma_start(out=ident_sb, in_=ident_ext[:, :])
            perm_f = singles.tile([P, P], f32)
            nc.scalar.dma_start(out=perm_f, in_=perm_ext[:, :])
            perm_sb = singles.tile([P, P], bf16)
            nc.vector.tensor_copy(out=perm_sb, in_=perm_f)
            one1 = singles.tile([1, 1], bf16)
            nc.vector.memset(one1, 1.0)

            wq_f = singles.tile([P, ND, H], f32)
            nc.scalar.dma_start(out=wq_f, in_=wq_ext.rearrange("(j p) h -> p j h", p=P))
            wk_f = singles.tile([P, ND, H], f32)
            nc.scalar.dma_start(out=wk_f, in_=wk_ext.rearrange("(j p) h -> p j h", p=P))
            wv_f = singles.tile([P, ND, H], f32)
            nc.scalar.dma_start(out=wv_f, in_=wv_ext.rearrange("(j p) h -> p j h", p=P))

            # Fused projection stationary: wA = [Wq/sqrt(H) | Wk] per d-chunk
            # -> proj psum rows 0:64 = qT/sqrt(H), rows 64:128 = kT.
            wA = singles.tile([P, ND, P], bf16)
            for j in range(ND):
                nc.vector.tensor_scalar(
                    out=wA[:, j, 0:H], in0=wq_f[:, j, :],
                    scalar1=inv_sqrt_h, scalar2=None, op0=mult,
                )
                nc.vector.tensor_copy(out=wA[:, j, H:P], in_=wk_f[:, j, :])
            wv_b = singles.tile([P, ND, H], bf16)
            for j in range(ND):
                nc.vector.tensor_copy(out=wv_b[:, j, :], in_=wv_f[:, j, :])

            # biasA: rows 0:64 = bq/sqrt(H), rows 64:128 = bk
            biasA = singles.tile([P, 1], f32)
            nc.scalar.dma_start(out=biasA[0:H, 0:1], in_=bq_ext[:, None])
            nc.scalar.dma_start(out=biasA[H:P, 0:1], in_=bk_ext[:, None])
            nc.vector.tensor_scalar(
                out=biasA[0:H, 0:1], in0=biasA[0:H, 0:1],
                scalar1=inv_sqrt_h, scalar2=None, op0=mult,
            )
            bv_sb = singles.tile([1, H], f32)
            nc.scalar.dma_start(out=bv_sb, in_=bv_ext[None, :])

            # Preload the exp ACT table while ScalarE is idle.
            tbl_dummy = singles.tile([P, 1], f32)
            nc.scalar.activation(out=tbl_dummy, in_=ident_sb[:, 0:1], func=Exp)

            # ---- PE warmup: keep HAM at K=8/8 during the DMA lead-in ----
            warm_ps = epi_pool.tile([P, 512], f32, tag="epi", name="warm")
            for wi in range(N_WARMUP_MM):
                nc.tensor.matmul(
                    warm_ps[:, 0:P], lhsT=perm_sb, rhs=perm_sb,
                    start=True, stop=True,
                )

            # ---- per-batch prologue state ----
            xn_tiles = [[None] * 4 for _ in range(BPC)]  # [b][k] -> [P, 4, 512]
            qkTA = [None] * BPC
            qkTB = [None] * BPC
            prev_last_transpose = None

            def cast_half(b, h, xs_b, hold=False):
                """cast HBM f32 -> SBUF xn bf16, bounce full rows to DRAM.
                Bounce writes go on the scalar HWDGE queue so the sync queue
                carries only transposes (no mode-switch interleave)."""
                nonlocal prev_last_transpose
                for k in (2 * h, 2 * h + 1):
                    xn_bk = xn_pool.tile([P, 4, 512], bf16, tag="xn",
                                         name=f"xn{b}_{k}")
                    xv = x_ext[b, 512 * k : 512 * (k + 1), :].rearrange(
                        "(t p) d -> p t d", p=P
                    )
                    ci = nc.gpsimd.dma_start(out=xn_bk, in_=xv)
                    if prev_last_transpose is not None:
                        add_dep_helper(
                            ci.ins, prev_last_transpose,
                            reason="hold casts until prior transpose group",
                        )
                    ov = xs_b[512 * k : 512 * (k + 1), :].rearrange(
                        "(t p) d -> p t d", p=P
                    )
                    nc.scalar.dma_start(out=ov, in_=xn_bk)
                    xn_tiles[b][k] = xn_bk

            def transpose_group(b, h, xs_b):
                """transpose one s-half (one DMA mode switch); xT per j."""
                nonlocal prev_last_transpose
                xTs = {}
                ti = None
                for j in range(ND):
                    xT_t = xT_pool.tile([P, 1024], bf16, tag="xT",
                                        name=f"xT{b}_{j}_{h}")
                    ti = nc.sync.dma_start_transpose(
                        out=xT_t,
                        in_=xs_b[1024 * h : 1024 * (h + 1), j * P : (j + 1) * P],
                    )
                    xTs[j] = xT_t
                prev_last_transpose = ti.ins
                return xTs

            def proj_chunks(b, cs, xTs, evac_engine):
                """projection chunks: all A matmuls first, then permute-B."""
                if qkTA[b] is None:
                    qkTA[b] = qkT_pool.tile([P, S], bf16, tag="qkT", name=f"qkTA{b}")
                    qkTB[b] = qkT_pool.tile([P, S], bf16, tag="qkT", name=f"qkTB{b}")
                A, Bt = qkTA[b], qkTB[b]
                for c in cs:
                    sl = slice(c * 512, (c + 1) * 512)
                    ssl = slice((c % 2) * 512, (c % 2) * 512 + 512)
                    pa = epi_pool.tile([P, 512], f32, tag="epi", name=f"pa{b}_{c}")
                    for j in range(ND):
                        nc.tensor.matmul(
                            pa, lhsT=wA[:, j, :], rhs=xTs[j][:, ssl],
                            start=(j == 0), stop=(j == ND - 1),
                        )
                    if evac_engine == "scalar":
                        nc.scalar.activation(
                            out=A[:, sl], in_=pa, func=Identity, bias=biasA[:, 0:1]
                        )
                    else:
                        nc.vector.tensor_scalar(
                            out=A[:, sl], in0=pa,
                            scalar1=biasA[:, 0:1], scalar2=None, op0=add,
                        )
                for c in cs:
                    sl = slice(c * 512, (c + 1) * 512)
                    pb = epi_pool.tile([P, 512], f32, tag="epi", name=f"pb{b}_{c}")
                    nc.tensor.matmul(pb, lhsT=perm_sb, rhs=A[:, sl],
                                     start=True, stop=True)
                    if evac_engine == "scalar":
                        nc.scalar.activation(out=Bt[:, sl], in_=pb, func=Copy)
                    else:
                        nc.vector.tensor_copy(out=Bt[:, sl], in_=pb)

            # Per-strip storage: scalar strips get a bf16 w tile; DVE strips
            # get an i16 tile (colsum reads it via bitcast).
            w_store = {0: [None] * NT, 1: [None] * NT}  # tile handle
            w_rhs = {0: [None] * NT, 1: [None] * NT}    # AP for colsum rhs

            def alloc_strip(b, i):
                if (16 * b + i) in DVE_TILES:
                    sct = sc_pool.tile([P, S], i16, tag="sc", name=f"sc{b}_{i}")
                    w_store[b][i] = sct
                    w_rhs[b][i] = sct.bitcast(bf16)
                else:
                    wt = w_pool.tile([P, S], bf16, tag="w", name=f"w{b}_{i}")
                    w_store[b][i] = wt
                    w_rhs[b][i] = wt

            # ---- scores + exp for one row strip ----
            def scores_tile(b, i, z_all, halves=(0, 1)):
                A, Bt = qkTA[b], qkTB[b]
                qsl = slice(i * P, (i + 1) * P)
                on_dve = (16 * b + i) in DVE_TILES
                wt = w_store[b][i]
                for half in halves:
                    c0, c1 = 2 * half, 2 * half + 1
                    ps = mm_pool.tile([P, 1024], f32, tag="mm",
                                      name=f"ps{b}_{i}_{half}")
                    # row-packed pair: rows 0:64 stream kT (from B low) for
                    # chunk c0; rows 64:128 stream kT (from A high) for c1.
                    nc.tensor.matmul(
                        ps[:, 0:512], lhsT=A[0:H, qsl],
                        rhs=Bt[0:H, c0 * 512 : (c0 + 1) * 512],
                        start=True, stop=True, tile_position=(0, 0),
                    )
                    nc.tensor.matmul(
                        ps[:, 512:1024], lhsT=Bt[H:P, qsl],
                        rhs=A[H:P, c1 * 512 : (c1 + 1) * 512],
                        start=True, stop=True, tile_position=(H, 0),
                    )
                    hs = slice(half * 1024, (half + 1) * 1024)
                    if on_dve:
                        # Schraudolph: affine to bf16 bit pattern as i16.
                        nc.vector.tensor_scalar(
                            out=wt[:, hs], in0=ps,
                            scalar1=A16, scalar2=B16, op0=mult, op1=add,
                        )
                    else:
                        nc.scalar.activation(
                            out=wt[:, hs], in_=ps, func=Exp,
                            accum_out=z_all[:, i, half : half + 1],
                        )
                if on_dve and 1 in halves:
                    # rowsum: byte-copy (SWDGE) to a real bf16 tile, then one
                    # in-place 4x pass with accum_out.
                    wbt = wb_pool.tile([P, S], bf16, tag="wb", name=f"wb{b}_{i}")
                    nc.gpsimd.dma_start(out=wbt, in_=wt.bitcast(bf16))
                    nc.vector.tensor_scalar(
                        out=wbt, in0=wbt, scalar1=1.0, scalar2=None,
                        op0=mult, op1=add, accum_out=z_all[:, i, 0:1],
                    )

            def colsum_group(b, g, z_all, rz_all, rzb_all, colbank):
                i0 = 4 * g
                nc.vector.reduce_sum(
                    out=rz_all[:, i0 : i0 + 4], in_=z_all[:, i0 : i0 + 4, :], axis=X
                )
                nc.vector.reciprocal(
                    out=rz_all[:, i0 : i0 + 4], in_=rz_all[:, i0 : i0 + 4]
                )
                nc.vector.tensor_copy(
                    out=rzb_all[:, i0 : i0 + 4], in_=rz_all[:, i0 : i0 + 4]
                )
                for i in range(i0, i0 + 4):
                    for c in range(NC4):
                        nc.tensor.matmul(
                            colbank[32 * c : 32 * c + 1, :],
                            lhsT=rzb_all[:, i : i + 1],
                            rhs=w_rhs[b][i][:, c * 512 : (c + 1) * 512],
                            start=(i == 0), stop=(i == NT - 1),
                            tile_position=(0, 32 * c),
                        )

            def epilogue(b, colbank):
                cbar_sb = misc_pool.tile([P, 512], f32, tag="cbar", name=f"cbar{b}")
                nc.vector.tensor_copy(out=cbar_sb, in_=colbank)
                cbT_ps = epi_pool.tile([P, 512], f32, tag="epi", name=f"cbT{b}")
                for f in range(4):
                    nc.tensor.transpose(
                        out=cbT_ps[:, f * P : (f + 1) * P],
                        in_=cbar_sb[:, f * P : (f + 1) * P],
                        identity=ident_sb,
                    )
                # good columns of cbT_ps are 128*f + 32*c -> tile index 4c+f
                cbT_sb = misc_pool.tile([P, 4, 4], bf16, tag="cbT", name=f"cbT{b}")
                src = cbT_ps[:, :].rearrange("p (f c r) -> p c f r", f=4, c=4, r=32)
                nc.vector.tensor_copy(out=cbT_sb, in_=src[:, :, :, 0])
                gp = epi_pool.tile([1, 512], f32, tag="epi", name=f"gp{b}")
                for t in range(NT):
                    nc.tensor.matmul(
                        gp, lhsT=cbT_sb[:, t // 4, t % 4 : t % 4 + 1],
                        rhs=xn_tiles[b][t // 4][:, t % 4, :],
                        start=(t == 0), stop=(t == NT - 1),
                    )
                g_sb = misc_pool.tile([1, D], bf16, tag="g", name=f"g{b}")
                nc.vector.tensor_copy(out=g_sb, in_=gp)
                gT_ps = epi_pool.tile([P, ND], f32, tag="epi", name=f"gT{b}")
                for j in range(ND):
                    nc.tensor.matmul(
                        gT_ps[:, j : j + 1], lhsT=g_sb[0:1, j * P : (j + 1) * P],
                        rhs=one1, start=True, stop=True,
                    )
                gT_sb = misc_pool.tile([P, ND], bf16, tag="gT", name=f"gTs{b}")
                nc.vector.tensor_copy(out=gT_sb, in_=gT_ps)
                fp = epi_pool.tile([1, H], f32, tag="epi", name=f"fp{b}")
                for j in range(ND):
                    nc.tensor.matmul(
                        fp, lhsT=gT_sb[:, j : j + 1], rhs=wv_b[:, j, :],
                        start=(j == 0), stop=(j == ND - 1),
                    )
                o_sb = misc_pool.tile([1, H], f32, tag="o", name=f"o{b}")
                nc.vector.scalar_tensor_tensor(
                    out=o_sb, in0=fp, scalar=1.0 / float(S), in1=bv_sb,
                    op0=mult, op1=add,
                )
                nc.scalar.dma_start(out=out_ext[b : b + 1, :], in_=o_sb)

            # ================= schedule =================
            xsd = {}
            for b in range(BPC):
                xsd[b] = dram_pool.tile([S, D], bf16, tag="xs", name=f"xs{b}")

            z_alls = {}
            rz_alls = {}
            rzb_alls = {}
            colbanks = {}
            for b in range(BPC):
                z_alls[b] = zr_pool.tile([P, NT, 2], f32, tag="z", name=f"z{b}")
                nc.vector.memset(z_alls[b], 0.0)
                rz_alls[b] = zr_pool.tile([P, NT], f32, tag="rz", name=f"rz{b}")
                rzb_alls[b] = zr_pool.tile([P, NT], bf16, tag="rzb", name=f"rzb{b}")

            def get_colbank(b):
                cb = col_pool.tile([P, 512], f32, tag="col", name=f"cb{b}")
                nc.vector.memset(cb, 0.0)
                colbanks[b] = cb

            # batch 0 prologue
            cast_half(0, 0, xsd[0])
            xT00 = transpose_group(0, 0, xsd[0])
            proj_chunks(0, (0, 1), xT00, "vector")
            cast_half(0, 1, xsd[0])
            xT01 = transpose_group(0, 1, xsd[0])

            get_colbank(0)
            za0 = z_alls[0]
            for i in range(8):
                alloc_strip(0, i)
                scores_tile(0, i, za0, halves=(0,))
            proj_chunks(0, (2, 3), xT01, "vector")
            for i in range(6):
                scores_tile(0, i, za0, halves=(1,))
                if i == 5:
                    colsum_group(0, 0, za0, rz_alls[0], rzb_alls[0], colbanks[0])

            cast_half(1, 0, xsd[1])
            cast_half(1, 1, xsd[1])

            for i in range(6, 8):
                scores_tile(0, i, za0, halves=(1,))
            for i in range(8, 12):
                alloc_strip(0, i)
                scores_tile(0, i, za0)
                if i == 9:
                    colsum_group(0, 1, za0, rz_alls[0], rzb_alls[0], colbanks[0])

            xT10 = transpose_group(1, 0, xsd[1])
            xT11 = transpose_group(1, 1, xsd[1])

            for i in range(12, NT):
                alloc_strip(0, i)
                scores_tile(0, i, za0)
                if i == 13:
                    colsum_group(0, 2, za0, rz_alls[0], rzb_alls[0], colbanks[0])

            proj_chunks(1, (0, 1), xT10, "vector")
            proj_chunks(1, (2, 3), xT11, "vector")

            colsum_group(0, 3, z_alls[0], rz_alls[0], rzb_alls[0], colbanks[0])
            epilogue(0, colbanks[0])

            get_colbank(1)
            za1 = z_alls[1]
            for i in range(8):
                alloc_strip(1, i)
                scores_tile(1, i, za1, halves=(0,))
            for i in range(8):
                scores_tile(1, i, za1, halves=(1,))
                if i == 5:
                    colsum_group(1, 0, za1, rz_alls[1], rzb_alls[1], colbanks[1])
            for i in range(8, NT):
                alloc_strip(1, i)
                scores_tile(1, i, za1)
                if i == 9:
                    colsum_group(1, 1, za1, rz_alls[1], rzb_alls[1], colbanks[1])
                if i == 13:
                    colsum_group(1, 2, za1, rz_alls[1], rzb_alls[1], colbanks[1])
            colsum_group(1, 3, z_alls[1], rz_alls[1], rzb_alls[1], colbanks[1])
            epilogue(1, colbanks[1])

    nc.finalize()
    return nc


_NC_CACHE = None


def _get_nc():
    global _NC_CACHE
    if _NC_CACHE is None:
        _NC_CACHE = build_nc()
    return _NC_CACHE


def run(inputs_map, trace=False, **spmd_kwargs):
    from concourse.bass_utils import run_bass_kernel_spmd

    x = np.ascontiguousarray(np.asarray(inputs_map["inputs"], dtype=np.float32))
    assert x.shape == (B, S, D), x.shape
    full = {
        "Wq": np.ascontiguousarray(np.asarray(inputs_map["Wq"], np.float32)),
        "bq": np.ascontiguousarray(np.asarray(inputs_map["bq"], np.float32)),
        "Wk": np.ascontiguousarray(np.asarray(inputs_map["Wk"], np.float32)),
        "bk": np.ascontiguousarray(np.asarray(inputs_map["bk"], np.float32)),
        "Wv": np.ascontiguousarray(np.asarray(inputs_map["Wv"], np.float32)),
        "bv": np.ascontiguousarray(np.asarray(inputs_map["bv"], np.float32)),
        "ident128": np.eye(P, dtype=np.float32),
        "perm64": np.roll(np.eye(P, dtype=np.float32), 64, axis=0),
    }
    in_maps = []
    for i in range(NCORES):
        m = {"inputs": np.ascontiguousarray(x[i * BPC : (i + 1) * BPC])}
        m.update(full)
        in_maps.append(m)
    nc = _get_nc()
    res = run_bass_kernel_spmd(
        nc, in_maps, core_ids=list(range(NCORES)), trace=trace, **spmd_kwargs
    )
    out = np.concatenate([np.asarray(res.results[i]["out"]) for i in range(NCORES)], 0)
    return out.astype(np.float32), res


def kernel(**inputs):
    out, _ = run(inputs, trace=False)
    return out


if __name__ == "__main__":
    rng = np.random.default_rng(0)
    ins = {
        "inputs": rng.standard_normal((B, S, D), dtype=np.float32),
        "Wq": rng.standard_normal((D, H), dtype=np.float32) / np.sqrt(D),
        "bq": np.zeros(H, np.float32),
        "Wk": rng.standard_normal((D, H), dtype=np.float32) / np.sqrt(D),
        "bk": np.zeros(H, np.float32),
        "Wv": rng.standard_normal((D, H), dtype=np.float32) / np.sqrt(D),
        "bv": np.zeros(H, np.float32),
    }
    out = kernel(**ins)
    print("out", out.shape, out[0, :4])


# revision 4
# speedup vs baseline: 1.2094x; 1.2094x over previous
"""Trainium2 Bass kernel for nn_AttentionLayer (B=16, S=2048, D=512, H=64).

Data-parallel over batch: 8 NeuronCores x 2 batch items each; no collectives.

Math (per batch item b):
  qT = (Wq^T x^T + bq)/sqrt(H);  kT = Wk^T x^T + bk      [64, S] each
  w = exp(qT^T kT)  (no rowmax pass: |scores| <= ~9)
  out[h] = sum_t cbar[t] * x[t, :] @ Wv / S + bv, cbar = sum_s w[s,:]/Z[s]
  (V is never materialized.)

Structure:
  - batch-0 prologue: SWDGE cast HBM f32 -> SBUF xn bf16, then xT via PE
    transposes (bf16 identity) evacuated by ScalarE (idle in prologue).
    No DRAM bounce for batch 0; PE transposes double as HAM warmup.
  - batch-1 prologue (overlapped under batch-0 strips): DRAM bounce +
    sync-queue DMA transposes (bounce writes on the scalar HWDGE queue so
    the sync queue carries only transposes).
  - fused projection stationary [Wq/sqrt(H) | Wk] -> A = [qT;kT];
    B = [kT;qT] via one PE permutation matmul per chunk.
  - scores row-strips [128,2048] via row-packed matmul pairs
    (tile_position (0,0)/(64,0)) streaming two column chunks at once.
  - exp SPLIT between ScalarE and VectorE per row-strip:
      * ScalarE strips: activation Exp (f32 PSUM -> bf16 SBUF) with
        accum_out rowsum.
      * DVE strips: Schraudolph bit-trick (i16 = s*log2e*128 + 16250.9 ==
        bf16 bits of exp(s)) via tensor_scalar f32 PSUM -> i16 SBUF.
        The colsum matmul reads the i16 tile DIRECTLY via .bitcast(bf16)
        (PE reads of bitcast APs are safe on HW - measured; DVE/ScalarE
        compute ops on bitcast APs crash). Rowsum: one SWDGE byte-copy to
        a real bf16 tile, one tensor_add fold 2048->1024 (2x mode), one
        reduce_sum (1x).  accum_out on DVE is avoided: its
        TENSOR_SCALAR_CACHE_REDUCE costs a full 1x pass (~2.3us).
      Per-row softmax normalization cancels the Schraudolph per-element
      bias exactly (each row lives entirely on one engine).
  - colsum in groups of 4 strips (batched reciprocal), col-packed
    (0,32c) accumulation into one psum bank.
  - epilogue split in two parts so batch-1 strips issue between them
    (keeps PE/ScalarE fed across the batch boundary).

Known landmines (measured on HW):
  - bitcast APs on DVE/ScalarE tensor ops -> NRT_EXEC_UNIT_UNRECOVERABLE.
    PE matmul rhs and DMA APs are safe.
  - SWDGE DRAM->DRAM cast DMA -> INTERNAL crash.
  - gpsimd tensor_scalar with accum_out fails to compile; accum_out
    needs op1 set; gpsimd cannot read PSUM or reduce along free axis.
  - dma_start_transpose needs contiguous last-dim DRAM input, 2-byte
    dtype (hence the DRAM bounce for batch-1 xT).
  - interleaving SBUF->SBUF copies with DMA transposes on the Sync queue
    causes multi-us mode-switch drains.
"""

import os as _os

import numpy as np

B, S, D, H = 16, 2048, 512, 64
NCORES = 8
BPC = B // NCORES  # batches per core
P = 128
NT = S // P  # 16 row strips
ND = D // P  # 4 d tiles
NC4 = S // 512  # 4 column chunks of 512

# Row strips whose exp runs on VectorE (Schraudolph); rest on ScalarE.
if _os.environ.get("K_DVE_EXP"):
    DVE_TILES = tuple(int(t) for t in _os.environ["K_DVE_EXP"].split(","))
else:
    _per = tuple(
        int(t) for t in _os.environ.get("K_DVE_PER", "2,5,8,11,14").split(",")
    )
    DVE_TILES = tuple(b * 16 + i for b in range(BPC) for i in _per)

# Schraudolph constants for bf16-bits exp: i16 = s * A16 + B16
A16 = float(np.log2(np.e) * 128.0)
B16 = 16250.91

N_WARMUP_MM = int(_os.environ.get("K_WARM", "48"))


def build_nc():
    import concourse.bacc as bacc
    import concourse.mybir as mybir
    import concourse.tile as tile

    f32 = mybir.dt.float32
    bf16 = mybir.dt.bfloat16
    i16 = mybir.dt.int16
    Exp = mybir.ActivationFunctionType.Exp
    Identity = mybir.ActivationFunctionType.Identity
    Copy = mybir.ActivationFunctionType.Copy
    X = mybir.AxisListType.X
    add = mybir.AluOpType.add
    mult = mybir.AluOpType.mult

    nc = bacc.Bacc("TRN2", target_bir_lowering=False)

    x_ext = nc.declare_dram_parameter("inputs", [BPC, S, D], f32, isOutput=False)
    wq_ext = nc.declare_dram_parameter("Wq", [D, H], f32, isOutput=False)
    bq_ext = nc.declare_dram_parameter("bq", [H], f32, isOutput=False)
    wk_ext = nc.declare_dram_parameter("Wk", [D, H], f32, isOutput=False)
    bk_ext = nc.declare_dram_parameter("bk", [H], f32, isOutput=False)
    wv_ext = nc.declare_dram_parameter("Wv", [D, H], f32, isOutput=False)
    bv_ext = nc.declare_dram_parameter("bv", [H], f32, isOutput=False)
    ident_ext = nc.declare_dram_parameter("ident128", [P, P], f32, isOutput=False)
    perm_ext = nc.declare_dram_parameter("perm64", [P, P], f32, isOutput=False)
    out_ext = nc.declare_dram_parameter("out", [BPC, H], f32, isOutput=True)

    inv_sqrt_h = 1.0 / float(np.sqrt(H))

    with tile.TileContext(nc) as tc:
        with (
            tc.tile_pool(name="singles", bufs=1) as singles,
            tc.tile_pool(name="xn", bufs=8) as xn_pool,
            tc.tile_pool(name="xT", bufs=16) as xT_pool,
            tc.tile_pool(name="qkT", bufs=4) as qkT_pool,
            tc.tile_pool(name="w", bufs=9) as w_pool,
            tc.tile_pool(name="sc16", bufs=4) as sc_pool,
            tc.tile_pool(name="wb", bufs=3) as wb_pool,
            tc.tile_pool(name="zr", bufs=6) as zr_pool,
            tc.tile_pool(name="misc", bufs=4) as misc_pool,
            tc.tile_pool(name="dram", bufs=4, space="DRAM") as dram_pool,
            tc.tile_pool(name="mm", bufs=3, space="PSUM") as mm_pool,
            tc.tile_pool(name="col", bufs=1, space="PSUM") as col_pool,
            tc.tile_pool(name="epi", bufs=1, space="PSUM") as epi_pool,
        ):
            # ---- constants / weights prep (once); scalar HWDGE queue ----
            ident_sb = singles.tile([P, P], f32)
            nc.scalar.dma_start(out=ident_sb, in_=ident_ext[:, :])
            perm_f = singles.tile([P, P], f32)
            nc.scalar.dma_start(out=perm_f, in_=perm_ext[:, :])
            perm_sb = singles.tile([P, P], bf16)
            nc.vector.tensor_copy(out=perm_sb, in_=perm_f)
            ident_b = singles.tile([P, P], bf16)
            nc.vector.tensor_copy(out=ident_b, in_=ident_sb)
            one1 = singles.tile([1, 1], bf16)
            nc.vector.memset(one1, 1.0)

            wq_f = singles.tile([P, ND, H], f32)
            nc.scalar.dma_start(out=wq_f, in_=wq_ext.rearrange("(j p) h -> p j h", p=P))
            wk_f = singles.tile([P, ND, H], f32)
            nc.scalar.dma_start(out=wk_f, in_=wk_ext.rearrange("(j p) h -> p j h", p=P))
            wv_f = singles.tile([P, ND, H], f32)
            nc.scalar.dma_start(out=wv_f, in_=wv_ext.rearrange("(j p) h -> p j h", p=P))

            # Fused projection stationary: wA = [Wq/sqrt(H) | Wk] per d-chunk
            # -> proj psum rows 0:64 = qT/sqrt(H), rows 64:128 = kT.
            wA = singles.tile([P, ND, P], bf16)
            for j in range(ND):
                nc.vector.tensor_scalar(
                    out=wA[:, j, 0:H], in0=wq_f[:, j, :],
                    scalar1=inv_sqrt_h, scalar2=None, op0=mult,
                )
                nc.vector.tensor_copy(out=wA[:, j, H:P], in_=wk_f[:, j, :])
            wv_b = singles.tile([P, ND, H], bf16)
            for j in range(ND):
                nc.vector.tensor_copy(out=wv_b[:, j, :], in_=wv_f[:, j, :])

            # biasA: rows 0:64 = bq/sqrt(H), rows 64:128 = bk
            biasA = singles.tile([P, 1], f32)
            nc.scalar.dma_start(out=biasA[0:H, 0:1], in_=bq_ext[:, None])
            nc.scalar.dma_start(out=biasA[H:P, 0:1], in_=bk_ext[:, None])
            nc.vector.tensor_scalar(
                out=biasA[0:H, 0:1], in0=biasA[0:H, 0:1],
                scalar1=inv_sqrt_h, scalar2=None, op0=mult,
            )
            bv_sb = singles.tile([1, H], f32)
            nc.scalar.dma_start(out=bv_sb, in_=bv_ext[None, :])

            # Preload the exp ACT table while ScalarE is idle.
            tbl_dummy = singles.tile([P, 1], f32)
            nc.scalar.activation(out=tbl_dummy, in_=ident_sb[:, 0:1], func=Exp)

            # ---- PE warmup: release the HAM clock gate during the lead-in
            warm_ps = epi_pool.tile([P, 512], f32, tag="epi", name="warm")
            for wi in range(N_WARMUP_MM):
                nc.tensor.matmul(
                    warm_ps[:, 0:P], lhsT=perm_sb, rhs=perm_sb,
                    start=True, stop=True,
                )

            # ---- per-batch prologue state ----
            xn_tiles = [[None] * 4 for _ in range(BPC)]  # [b][k] -> [P, 4, 512]
            qkTA = [None] * BPC
            qkTB = [None] * BPC
            prev_last_transpose = None

            def cast_quarters(b, ks, xs_b=None):
                """cast HBM f32 -> SBUF xn bf16 (SWDGE); optionally bounce
                full rows to DRAM on the scalar HWDGE queue (batch 1)."""
                for k in ks:
                    xn_bk = xn_pool.tile([P, 4, 512], bf16, tag="xn",
                                         name=f"xn{b}_{k}")
                    xv = x_ext[b, 512 * k : 512 * (k + 1), :].rearrange(
                        "(t p) d -> p t d", p=P
                    )
                    nc.gpsimd.dma_start(out=xn_bk, in_=xv)
                    if xs_b is not None:
                        ov = xs_b[512 * k : 512 * (k + 1), :].rearrange(
                            "(t p) d -> p t d", p=P
                        )
                        nc.scalar.dma_start(out=ov, in_=xn_bk)
                    xn_tiles[b][k] = xn_bk

            def pe_transpose_half(b, h):
                """xT for one s-half via PE transposes + ScalarE evac."""
                xTs = {}
                for j in range(ND):
                    tp = mm_pool.tile([P, 1024], bf16, tag="mm",
                                      name=f"tp{b}_{j}_{h}")
                    for u in range(8):
                        k = 2 * h + u // 4
                        tt = u % 4
                        nc.tensor.transpose(
                            out=tp[:, u * P : (u + 1) * P],
                            in_=xn_tiles[b][k][:, tt, j * P : (j + 1) * P],
                            identity=ident_b,
                        )
                    xT_t = xT_pool.tile([P, 1024], bf16, tag="xT",
                                        name=f"xT{b}_{j}_{h}")
                    nc.scalar.activation(out=xT_t, in_=tp, func=Copy)
                    xTs[j] = xT_t
                return xTs

            def transpose_group(b, h, xs_b):
                """batch-1 path: transpose one s-half from the DRAM bounce."""
                nonlocal prev_last_transpose
                xTs = {}
                ti = None
                for j in range(ND):
                    xT_t = xT_pool.tile([P, 1024], bf16, tag="xT",
                                        name=f"xT{b}_{j}_{h}")
                    ti = nc.sync.dma_start_transpose(
                        out=xT_t,
                        in_=xs_b[1024 * h : 1024 * (h + 1), j * P : (j + 1) * P],
                    )
                    xTs[j] = xT_t
                prev_last_transpose = ti.ins
                return xTs

            def proj_chunks(b, cs, xTs, evac_engine):
                """projection chunks: all A matmuls first, then permute-B."""
                if qkTA[b] is None:
                    qkTA[b] = qkT_pool.tile([P, S], bf16, tag="qkT", name=f"qkTA{b}")
                    qkTB[b] = qkT_pool.tile([P, S], bf16, tag="qkT", name=f"qkTB{b}")
                A, Bt = qkTA[b], qkTB[b]
                for c in cs:
                    sl = slice(c * 512, (c + 1) * 512)
                    ssl = slice((c % 2) * 512, (c % 2) * 512 + 512)
                    pa = epi_pool.tile([P, 512], f32, tag="epi", name=f"pa{b}_{c}")
                    for j in range(ND):
                        nc.tensor.matmul(
                            pa, lhsT=wA[:, j, :], rhs=xTs[j][:, ssl],
                            start=(j == 0), stop=(j == ND - 1),
                        )
                    if evac_engine == "scalar":
                        nc.scalar.activation(
                            out=A[:, sl], in_=pa, func=Identity, bias=biasA[:, 0:1]
                        )
                    else:
                        nc.vector.tensor_scalar(
                            out=A[:, sl], in0=pa,
                            scalar1=biasA[:, 0:1], scalar2=None, op0=add,
                        )
                for c in cs:
                    sl = slice(c * 512, (c + 1) * 512)
                    pb = epi_pool.tile([P, 512], f32, tag="epi", name=f"pb{b}_{c}")
                    nc.tensor.matmul(pb, lhsT=perm_sb, rhs=A[:, sl],
                                     start=True, stop=True)
                    if evac_engine == "scalar":
                        nc.scalar.activation(out=Bt[:, sl], in_=pb, func=Copy)
                    else:
                        nc.vector.tensor_copy(out=Bt[:, sl], in_=pb)

            # Per-strip storage: scalar strips get a bf16 w tile; DVE strips
            # an i16 tile (colsum reads it via bitcast).
            w_store = {0: [None] * NT, 1: [None] * NT}
            w_rhs = {0: [None] * NT, 1: [None] * NT}

            def alloc_strip(b, i):
                if (16 * b + i) in DVE_TILES:
                    sct = sc_pool.tile([P, S], i16, tag="sc", name=f"sc{b}_{i}")
                    w_store[b][i] = sct
                    w_rhs[b][i] = sct.bitcast(bf16)
                else:
                    wt = w_pool.tile([P, S], bf16, tag="w", name=f"w{b}_{i}")
                    w_store[b][i] = wt
                    w_rhs[b][i] = wt

            # ---- scores + exp for one row strip ----
            def scores_tile(b, i, z_all, halves=(0, 1)):
                A, Bt = qkTA[b], qkTB[b]
                qsl = slice(i * P, (i + 1) * P)
                on_dve = (16 * b + i) in DVE_TILES
                wt = w_store[b][i]
                for half in halves:
                    c0, c1 = 2 * half, 2 * half + 1
                    ps = mm_pool.tile([P, 1024], f32, tag="mm",
                                      name=f"ps{b}_{i}_{half}")
                    nc.tensor.matmul(
                        ps[:, 0:512], lhsT=A[0:H, qsl],
                        rhs=Bt[0:H, c0 * 512 : (c0 + 1) * 512],
                        start=True, stop=True, tile_position=(0, 0),
                    )
                    nc.tensor.matmul(
                        ps[:, 512:1024], lhsT=Bt[H:P, qsl],
                        rhs=A[H:P, c1 * 512 : (c1 + 1) * 512],
                        start=True, stop=True, tile_position=(H, 0),
                    )
                    hs = slice(half * 1024, (half + 1) * 1024)
                    if on_dve:
                        nc.vector.tensor_scalar(
                            out=wt[:, hs], in0=ps,
                            scalar1=A16, scalar2=B16, op0=mult, op1=add,
                        )
                    else:
                        nc.scalar.activation(
                            out=wt[:, hs], in_=ps, func=Exp,
                            accum_out=z_all[:, i, half : half + 1],
                        )
                if on_dve and 1 in halves:
                    # rowsum: byte-copy to a real bf16 tile, fold 2048->1024
                    # (2x tensor_add), then one 1x reduce.
                    wbt = wb_pool.tile([P, S], bf16, tag="wb", name=f"wb{b}_{i}")
                    nc.gpsimd.dma_start(out=wbt, in_=wt.bitcast(bf16))
                    nc.vector.tensor_add(
                        out=wbt[:, 0:1024], in0=wbt[:, 0:1024],
                        in1=wbt[:, 1024:2048],
                    )
                    nc.vector.reduce_sum(
                        out=z_all[:, i, 0:1], in_=wbt[:, 0:1024], axis=X
                    )

            def colsum_group(b, g, z_all, rz_all, rzb_all, colbank):
                i0 = 4 * g
                nc.vector.reduce_sum(
                    out=rz_all[:, i0 : i0 + 4], in_=z_all[:, i0 : i0 + 4, :], axis=X
                )
                nc.vector.reciprocal(
                    out=rz_all[:, i0 : i0 + 4], in_=rz_all[:, i0 : i0 + 4]
                )
                nc.vector.tensor_copy(
                    out=rzb_all[:, i0 : i0 + 4], in_=rz_all[:, i0 : i0 + 4]
                )
                for i in range(i0, i0 + 4):
                    for c in range(NC4):
                        nc.tensor.matmul(
                            colbank[32 * c : 32 * c + 1, :],
                            lhsT=rzb_all[:, i : i + 1],
                            rhs=w_rhs[b][i][:, c * 512 : (c + 1) * 512],
                            start=(i == 0), stop=(i == NT - 1),
                            tile_position=(0, 32 * c),
                        )

            def epilogue_a(b, colbank):
                """cbar evac + transpose + extract (DVE/PE round trip)."""
                cbar_sb = misc_pool.tile([P, 512], f32, tag="cbar", name=f"cbar{b}")
                nc.vector.tensor_copy(out=cbar_sb, in_=colbank)
                cbT_ps = epi_pool.tile([P, 512], f32, tag="epi", name=f"cbT{b}")
                for f in range(4):
                    nc.tensor.transpose(
                        out=cbT_ps[:, f * P : (f + 1) * P],
                        in_=cbar_sb[:, f * P : (f + 1) * P],
                        identity=ident_sb,
                    )
                cbT_sb = misc_pool.tile([P, 4, 4], bf16, tag="cbT", name=f"cbT{b}")
                src = cbT_ps[:, :].rearrange("p (f c r) -> p c f r", f=4, c=4, r=32)
                nc.vector.tensor_copy(out=cbT_sb, in_=src[:, :, :, 0])
                return cbT_sb

            def epilogue_b(b, cbT_sb):
                gp = epi_pool.tile([1, 512], f32, tag="epi", name=f"gp{b}")
                for t in range(NT):
                    nc.tensor.matmul(
                        gp, lhsT=cbT_sb[:, t // 4, t % 4 : t % 4 + 1],
                        rhs=xn_tiles[b][t // 4][:, t % 4, :],
                        start=(t == 0), stop=(t == NT - 1),
                    )
                g_sb = misc_pool.tile([1, D], bf16, tag="g", name=f"g{b}")
                nc.vector.tensor_copy(out=g_sb, in_=gp)
                gT_ps = epi_pool.tile([P, ND], f32, tag="epi", name=f"gT{b}")
                for j in range(ND):
                    nc.tensor.matmul(
                        gT_ps[:, j : j + 1], lhsT=g_sb[0:1, j * P : (j + 1) * P],
                        rhs=one1, start=True, stop=True,
                    )
                gT_sb = misc_pool.tile([P, ND], bf16, tag="gT", name=f"gTs{b}")
                nc.vector.tensor_copy(out=gT_sb, in_=gT_ps)
                fp = epi_pool.tile([1, H], f32, tag="epi", name=f"fp{b}")
                for j in range(ND):
                    nc.tensor.matmul(
                        fp, lhsT=gT_sb[:, j : j + 1], rhs=wv_b[:, j, :],
                        start=(j == 0), stop=(j == ND - 1),
                    )
                o_sb = misc_pool.tile([1, H], f32, tag="o", name=f"o{b}")
                nc.vector.scalar_tensor_tensor(
                    out=o_sb, in0=fp, scalar=1.0 / float(S), in1=bv_sb,
                    op0=mult, op1=add,
                )
                nc.scalar.dma_start(out=out_ext[b : b + 1, :], in_=o_sb)

            # ================= schedule =================
            xsd1 = dram_pool.tile([S, D], bf16, tag="xs", name="xs1")

            z_alls = {}
            rz_alls = {}
            rzb_alls = {}
            colbanks = {}
            for b in range(BPC):
                z_alls[b] = zr_pool.tile([P, NT, 2], f32, tag="z", name=f"z{b}")
                nc.vector.memset(z_alls[b], 0.0)
                rz_alls[b] = zr_pool.tile([P, NT], f32, tag="rz", name=f"rz{b}")
                rzb_alls[b] = zr_pool.tile([P, NT], bf16, tag="rzb", name=f"rzb{b}")

            def get_colbank(b):
                cb = col_pool.tile([P, 512], f32, tag="col", name=f"cb{b}")
                nc.vector.memset(cb, 0.0)
                colbanks[b] = cb

            # batch 0 prologue: casts up-front, PE transposes, no bounce
            cast_quarters(0, (0, 1))
            cast_quarters(0, (2, 3))
            xT00 = pe_transpose_half(0, 0)
            proj_chunks(0, (0, 1), xT00, "vector")
            xT01 = pe_transpose_half(0, 1)

            get_colbank(0)
            za0 = z_alls[0]
            for i in range(8):
                alloc_strip(0, i)
                scores_tile(0, i, za0, halves=(0,))
            proj_chunks(0, (2, 3), xT01, "vector")
            for i in range(6):
                scores_tile(0, i, za0, halves=(1,))
                if i == 5:
                    colsum_group(0, 0, za0, rz_alls[0], rzb_alls[0], colbanks[0])

            cast_quarters(1, (0, 1), xsd1)
            cast_quarters(1, (2, 3), xsd1)

            for i in range(6, 8):
                scores_tile(0, i, za0, halves=(1,))
            for i in range(8, 12):
                alloc_strip(0, i)
                scores_tile(0, i, za0)
                if i == 9:
                    colsum_group(0, 1, za0, rz_alls[0], rzb_alls[0], colbanks[0])

            xT10 = transpose_group(1, 0, xsd1)
            xT11 = transpose_group(1, 1, xsd1)

            for i in range(12, NT):
                alloc_strip(0, i)
                scores_tile(0, i, za0)
                if i == 13:
                    colsum_group(0, 2, za0, rz_alls[0], rzb_alls[0], colbanks[0])

            proj_chunks(1, (0, 1), xT10, "vector")
            proj_chunks(1, (2, 3), xT11, "vector")

            colsum_group(0, 3, z_alls[0], rz_alls[0], rzb_alls[0], colbanks[0])
            cbT0 = epilogue_a(0, colbanks[0])

            # start batch-1 strips between the two epilogue parts so PE and
            # ScalarE stay fed across the batch boundary
            get_colbank(1)
            za1 = z_alls[1]
            for i in range(8):
                alloc_strip(1, i)
                scores_tile(1, i, za1, halves=(0,))

            epilogue_b(0, cbT0)

            for i in range(8):
                scores_tile(1, i, za1, halves=(1,))
                if i == 5:
                    colsum_group(1, 0, za1, rz_alls[1], rzb_alls[1], colbanks[1])
            for i in range(8, NT):
                alloc_strip(1, i)
                scores_tile(1, i, za1)
                if i == 9:
                    colsum_group(1, 1, za1, rz_alls[1], rzb_alls[1], colbanks[1])
                if i == 13:
                    colsum_group(1, 2, za1, rz_alls[1], rzb_alls[1], colbanks[1])
            colsum_group(1, 3, z_alls[1], rz_alls[1], rzb_alls[1], colbanks[1])
            cbT1 = epilogue_a(1, colbanks[1])
            epilogue_b(1, cbT1)

    nc.finalize()
    return nc


_NC_CACHE = None


def _get_nc():
    global _NC_CACHE
    if _NC_CACHE is None:
        _NC_CACHE = build_nc()
    return _NC_CACHE


def run(inputs_map, trace=False, **spmd_kwargs):
    from concourse.bass_utils import run_bass_kernel_spmd

    x = np.ascontiguousarray(np.asarray(inputs_map["inputs"], dtype=np.float32))
    assert x.shape == (B, S, D), x.shape
    full = {
        "Wq": np.ascontiguousarray(np.asarray(inputs_map["Wq"], np.float32)),
        "bq": np.ascontiguousarray(np.asarray(inputs_map["bq"], np.float32)),
        "Wk": np.ascontiguousarray(np.asarray(inputs_map["Wk"], np.float32)),
        "bk": np.ascontiguousarray(np.asarray(inputs_map["bk"], np.float32)),
        "Wv": np.ascontiguousarray(np.asarray(inputs_map["Wv"], np.float32)),
        "bv": np.ascontiguousarray(np.asarray(inputs_map["bv"], np.float32)),
        "ident128": np.eye(P, dtype=np.float32),
        "perm64": np.roll(np.eye(P, dtype=np.float32), 64, axis=0),
    }
    in_maps = []
    for i in range(NCORES):
        m = {"inputs": np.ascontiguousarray(x[i * BPC : (i + 1) * BPC])}
        m.update(full)
        in_maps.append(m)
    nc = _get_nc()
    res = run_bass_kernel_spmd(
        nc, in_maps, core_ids=list(range(NCORES)), trace=trace, **spmd_kwargs
    )
    out = np.concatenate([np.asarray(res.results[i]["out"]) for i in range(NCORES)], 0)
    return out.astype(np.float32), res


def kernel(**inputs):
    out, _ = run(inputs, trace=False)
    return out


if __name__ == "__main__":
    rng = np.random.default_rng(0)
    ins = {
        "inputs": rng.standard_normal((B, S, D), dtype=np.float32),
        "Wq": rng.standard_normal((D, H), dtype=np.float32) / np.sqrt(D),
        "bq": np.zeros(H, np.float32),
        "Wk": rng.standard_normal((D, H), dtype=np.float32) / np.sqrt(D),
        "bk": np.zeros(H, np.float32),
        "Wv": rng.standard_normal((D, H), dtype=np.float32) / np.sqrt(D),
        "bv": np.zeros(H, np.float32),
    }
    out = kernel(**ins)
    print("out", out.shape, out[0, :4])
